# revision 31
# baseline (speedup 1.0000x reference)
"""GCN denoise net (2-layer GCNConv + time MLP) on 8 Trainium2 NeuronCores.

Strategy (v4 — unified piece schedule, fp8 masks, separable-norm folding):
  - Aggregate-then-transform: out = (A_hat @ x) @ W.T + b, exploiting linearity.
  - A_hat = D^-1/2 (A+I) D^-1/2 is separable: dinv[src] is folded into the
    gathered token table (host pre-scales), dinv[tgt] is applied once per
    target window after aggregation.  Edge "stiles" are pure 0/1 one-hots
    streamed as fp8 (exact).
  - ONE schedule for both layers: edges grouped by (source piece, target
    window), where pieces are the h1 AllGather pieces (windows [0,24) and
    [24,49) of every core).  The z table is laid out in the SAME piece-major
    row order the AllGather produces, so layer 1 and layer 2 share one fp8
    mask stream and one gather-index stream; only the gather source tensor
    differs (z table vs gathered h1).
  - Nodes permuted into 50176 "token" slots (392 windows of 128) with a
    degree-balanced serpentine assignment; per-(piece, window) edge groups
    fit Ma/Mb blocks of 128.
  - Targets sharded: core c owns windows [c*49, (c+1)*49).
  - Edge aggregation per 128-target window via PE one-hot matmuls
    (f16 gathered features x fp8 masks, accumulated in PSUM).
  - Source features fetched by gpsimd dma_gather (1024-idx calls, HW max);
    self-loops are one dense matmul per window (identity diag) against the
    own-shard tile.
  - temb (time-MLP table, b1 folded) preloaded once in wrapped layout.
  - h1 (dinv-scaled on-chip) exchanged between layers with a two-piece
    on-device AllGather; own h1 windows stay resident in SBUF for layer-2
    self matmuls.
"""

import sys
import numpy as np

sys.path.insert(0, "/opt/trn_rl_repo")

# ---------------- problem constants (hardcoded per contract) ----------------
N = 50000
E = 640000
D = 128
CORES = 8
NW = 392                      # 128-token windows total
NTOK = NW * 128               # 50176 padded tokens
WPC = NW // CORES             # 49 windows per core
SHARD = WPC * 128             # 6272 tokens per core
G = 8                         # gather chunk size in blocks (1024 idxs = HW max)
TMAX = 1000
PB = [0, 24, WPC]             # AllGather piece boundaries (windows per core);
                              # both pieces must stay <= 31 windows for int16
P0ROWS = CORES * PB[1] * 128  # 24576 rows in piece 0


def _prep(z, edge_index, t):
    """Host preprocessing: degrees, dinv, balanced permutation, one shared
    (piece, window) edge schedule, fp8 one-hot masks."""
    import ml_dtypes
    f8 = ml_dtypes.float8_e4m3

    row = np.asarray(edge_index[0], dtype=np.int64)
    col = np.asarray(edge_index[1], dtype=np.int64)

    deg = np.bincount(col, minlength=N).astype(np.float64) + 1.0
    dinv = (1.0 / np.sqrt(deg)).astype(np.float32)

    # --- balanced node->token permutation (serpentine over in-degree) ---
    indeg = np.bincount(col, minlength=N)
    order = np.argsort(-indeg, kind="stable")
    i = np.arange(N)
    r = i // NW
    j = i % NW
    win = np.where(r % 2 == 0, j, NW - 1 - j)
    token = win * 128 + r
    tok = np.empty(N, dtype=np.int64)
    tok[order] = token
    inv_tok = np.full(NTOK, -1, dtype=np.int64)
    inv_tok[tok] = np.arange(N)

    tr = tok[row]
    tc = tok[col]

    # piece-major row index of each source token (matches AllGather layout):
    # piece0: rows k*3072 + wl*128 + off (wl < 24), piece1 relative rows
    k_ = tr // SHARD
    wl_ = (tr % SHARD) >> 7
    off_ = tr & 127
    src_piece = (wl_ >= PB[1]).astype(np.int64)
    pw_ = np.where(src_piece == 0, PB[1], WPC - PB[1])
    src_row = k_ * (pw_ * 128) + (wl_ - src_piece * PB[1]) * 128 + off_

    core_of = tc // SHARD
    per_core = []
    Ma = 1
    Mb = 1
    for k in range(CORES):
        sel = core_of == k
        tck = tc[sel]
        wloc = (tck - k * SHARD) >> 7
        coff = tck & 127
        key = src_piece[sel] * WPC + wloc
        o2 = np.argsort(key, kind="stable")
        sidx = src_row[sel][o2]
        coffo = coff[o2]
        cnt = np.bincount(key[o2], minlength=2 * WPC)
        Ma = max(Ma, int(np.ceil(cnt[:WPC].max() / 128)))
        Mb = max(Mb, int(np.ceil(cnt[WPC:].max() / 128)))
        per_core.append((sidx, coffo, cnt))

    NBT = WPC * (Ma + Mb)     # total blocks in the shared schedule
    # per-group valid-idx count, uniform across cores (goes in num_idxs_reg):
    # cores with fewer edges pad with zero-row idxs up to Pg, then -1 beyond
    Pg = np.max(np.stack([pc[2] for pc in per_core]), axis=0)   # [2*WPC]

    core_inputs = []
    for k in range(CORES):
        sidx, coff, cnt = per_core[k]
        # pad slots get idx -1: they trail within each (piece, window) group,
        # and gather calls are group-aligned, so the gather ucode trims them
        # before descriptor generation (no Q7/SDMA cost).
        s_arr = np.full(NBT * 128, -1, dtype=np.int16)
        c_arr = np.full(NBT * 128, -1, dtype=np.int64)
        starts = np.concatenate([[0], np.cumsum(cnt)])
        for g in range(2 * WPC):
            m = int(cnt[g])
            if g < WPC:
                b0 = g * Ma * 128
            else:
                b0 = (WPC * Ma + (g - WPC) * Mb) * 128
            if m > 0:
                src = slice(starts[g], starts[g] + m)
                s_arr[b0:b0 + m] = sidx[src].astype(np.int16)
                c_arr[b0:b0 + m] = coff[src]
            # zero-row pads up to the shared per-group count Pg[g]
            s_arr[b0 + m:b0 + int(Pg[g])] = 0

        # transposed fp8 0/1 masks: maskT[e, b*128 + t] = (coff(slot b,e)==t)
        st_flat = np.zeros((NBT * 128, 128), dtype=np.float32)
        val = c_arr >= 0
        st_flat[np.nonzero(val)[0], c_arr[val]] = 1.0
        maskT = np.ascontiguousarray(
            st_flat.reshape(NBT, 128, 128).transpose(1, 0, 2)
            .reshape(128, NBT * 128)).astype(f8)

        # wrapped gather-index layout: unwrapped[i] = tile[i % 16, i // 16],
        # replicated across the 8 groups of 16 partitions
        wr = np.ascontiguousarray(s_arr.reshape(NBT * 8, 16).T)  # [16, NBT*8]
        idx_t = np.ascontiguousarray(np.tile(wr, (8, 1)))        # [128, NBT*8]

        # self-loop identity diagonal [r, w*128+t]: 1.0 at own real tokens
        own = inv_tok[k * SHARD:(k + 1) * SHARD]                 # [6272]
        ok = own >= 0
        dg = np.where(ok, 1.0, 0.0).astype(np.float16)
        diagT = np.zeros((128, SHARD), dtype=np.float16)
        p = np.arange(SHARD)
        diagT[p & 127, (p >> 7) * 128 + (p & 127)] = dg

        # own z rows, dinv-scaled, wrapped: zself[p, w*128+f]
        zself = np.zeros((128, SHARD), dtype=np.float16)
        zs = np.zeros((SHARD, D), dtype=np.float16)
        zs[ok] = (z[own[ok]] * dinv[own[ok]][:, None]).astype(np.float16)
        zself[:, :] = zs.reshape(WPC, 128, D).transpose(1, 0, 2).reshape(128, SHARD)

        # dinvA[p, w*128+f] = dinv[token(w*128+p)]  (h1 pre-scale, bcast on f)
        dv = np.zeros(SHARD, dtype=np.float32)
        dv[ok] = dinv[own[ok]]
        dinvA = np.ascontiguousarray(
            dv.reshape(WPC, 128, 1).repeat(D, axis=2)
            .transpose(1, 0, 2).reshape(128, SHARD)).astype(np.float16)
        # dinvB[p, w*128+t] = dinv[token(w*128+t)]  (agg post-scale, bcast on p)
        dinvB = np.ascontiguousarray(
            np.broadcast_to(dv, (128, SHARD))).astype(np.float16)

        core_inputs.append({
            "maskT": maskT,
            "idx16": idx_t,
            "diagT": diagT,
            "zself": zself,
            "dinvA": dinvA,
            "dinvB": dinvB,
            "_own": own,
        })
    return (Ma, Mb, tuple(int(x) for x in Pg)), tok, inv_tok, dinv, core_inputs


_BUILD_CACHE = {}
LAST_RESULT = None


def _build(Ms):
    """Build the SPMD Bass program (one NEFF shared by all 8 cores)."""
    Ma, Mb, Pg = Ms
    import os as _os
    no_cc = bool(int(_os.environ.get("KERNEL_NO_CC", "0")))
    key = (Ma, Mb, Pg, no_cc)
    if key in _BUILD_CACHE:
        return _BUILD_CACHE[key]

    import concourse.bass as bass
    import concourse.mybir as mybir
    import concourse.tile as tile
    from concourse.bass import ts

    f8 = mybir.dt.float8e4
    f16 = mybir.dt.float16
    f32 = mybir.dt.float32
    i16 = mybir.dt.int16
    AF = mybir.ActivationFunctionType
    OP = mybir.AluOpType

    NBT = WPC * (Ma + Mb)

    from concourse import bacc
    nc = bacc.Bacc(num_devices=CORES, num_swdge_queues=4)

    # z table in piece-major row order (same layout the AllGather produces)
    ztp_d = nc.dram_tensor("ztp", [NTOK, D], f16, kind="ExternalInput")
    idx_d = nc.dram_tensor("idx16", [128, NBT * 8], i16, kind="ExternalInput")
    masks_d = nc.dram_tensor("maskT", [128, NBT * 128], f8, kind="ExternalInput")
    temb_d = nc.dram_tensor("temb", [128, SHARD], f16, kind="ExternalInput")
    diag_d = nc.dram_tensor("diagT", [128, SHARD], f16, kind="ExternalInput")
    zself_d = nc.dram_tensor("zself", [128, SHARD], f16, kind="ExternalInput")
    dinvA_d = nc.dram_tensor("dinvA", [128, SHARD], f16, kind="ExternalInput")
    dinvB_d = nc.dram_tensor("dinvB", [128, SHARD], f16, kind="ExternalInput")
    w1t_d = nc.dram_tensor("w1t", [D, D], f16, kind="ExternalInput")
    w2t_d = nc.dram_tensor("w2t", [D, D], f16, kind="ExternalInput")
    bias2_d = nc.dram_tensor("bias2", [1, D], f16, kind="ExternalInput")
    ones_d = nc.dram_tensor("ones", [1, D], f16, kind="ExternalInput")

    out_d = nc.dram_tensor("outshard", [SHARD, D], f16, kind="ExternalOutput")
    h1sp_d = [nc.dram_tensor(f"h1sp{q}", [(PB[q + 1] - PB[q]) * 128, D], f16)
              for q in range(2)]
    h1pf_d = [nc.dram_tensor(f"h1pf{q}", [CORES * (PB[q + 1] - PB[q]) * 128, D],
                             f16, addr_space="Shared") for q in range(2)]

    with tile.TileContext(nc) as tc:
        with (
            tc.tile_pool(name="const", bufs=1) as constp,
            tc.tile_pool(name="work", bufs=2) as workp,
            tc.tile_pool(name="psum", bufs=2, space="PSUM") as psump,
        ):
            def load_const(dram, shape, dtype, eng=None):
                t_ = constp.tile(shape, dtype, name=dram.name + "_t")
                (eng or nc.sync).dma_start(out=t_[:], in_=dram[:, :])
                return t_

            idx_t = constp.tile([128, NBT * 8], i16, name="idx16_t")
            _ha = WPC * Ma * 8
            nc.sync.dma_start(out=idx_t[:, 0:_ha], in_=idx_d[:, 0:_ha])
            nc.scalar.dma_start(out=idx_t[:, _ha:NBT * 8],
                                in_=idx_d[:, _ha:NBT * 8])
            w1t_t = load_const(w1t_d, [D, D], f16)
            w2t_t = load_const(w2t_d, [D, D], f16)
            bias2_t = load_const(bias2_d, [1, D], f16)
            ones_t = load_const(ones_d, [1, D], f16)
            temb_t = load_const(temb_d, [128, SHARD], f16)
            diag_t = load_const(diag_d, [128, SHARD], f16)
            zself_t = load_const(zself_d, [128, SHARD], f16)
            dinvA_t = load_const(dinvA_d, [128, SHARD], f16)
            dinvB_t = load_const(dinvB_d, [128, SHARD], f16)

            aggbuf = constp.tile([128, SHARD], f32, name="aggbuf")
            h1stage = constp.tile([128, SHARD], f16, name="h1stage")

            def transform(L, w, aggT):
                # aggT: [f, t] f16; hp = aggT.T @ W.T -> [t, f']
                hp = psump.tile([128, D], f32, tag="hp", name="hp")
                wm = w1t_t if L == 0 else w2t_t
                nc.tensor.matmul(hp[:], lhsT=aggT[:], rhs=wm[:],
                                 start=True, stop=(L == 0))
                if L == 0:
                    # x = conv + temb ; h1 = elu(x) = relu(x)+exp(min(x,0))-1
                    x = workp.tile([128, D], f32, tag="x", name="x")
                    nc.vector.tensor_tensor(out=x[:], in0=hp[:],
                                            in1=temb_t[:, ts(w, D)], op=OP.add)
                    mn = workp.tile([128, D], f32, tag="mn", name="mn")
                    nc.vector.tensor_scalar(out=mn[:], in0=x[:],
                                            scalar1=0.0, scalar2=-60.0,
                                            op0=OP.min, op1=OP.max)
                    ex = workp.tile([128, D], f16, tag="ex", name="ex")
                    nc.scalar.activation(ex[:], mn[:], AF.Exp)
                    rl = workp.tile([128, D], f16, tag="rl", name="rl")
                    nc.scalar.activation(rl[:], x[:], AF.Relu)
                    e1 = workp.tile([128, D], f16, tag="e1", name="e1")
                    nc.vector.tensor_scalar(out=e1[:], in0=ex[:],
                                            scalar1=1.0, scalar2=None,
                                            op0=OP.subtract)
                    h1r = workp.tile([128, D], f16, tag="h1r", name="h1r")
                    nc.vector.tensor_tensor(out=h1r[:], in0=e1[:], in1=rl[:],
                                            op=OP.add)
                    # store dinv-scaled h1 (both for gathers and self matmuls)
                    nc.vector.tensor_tensor(out=h1stage[:, ts(w, D)],
                                            in0=h1r[:],
                                            in1=dinvA_t[:, ts(w, D)],
                                            op=OP.mult)
                    p_ = 0 if w < PB[1] else 1
                    lw = w - PB[p_]
                    nc.sync.dma_start(
                        out=h1sp_d[p_][lw * 128:(lw + 1) * 128, :],
                        in_=h1stage[:, ts(w, D)])
                    if w == PB[p_ + 1] - 1:
                        if no_cc:
                            rows_ = (PB[p_ + 1] - PB[p_]) * 128
                            nc.sync.dma_start(out=h1pf_d[p_][0:rows_, :],
                                              in_=h1sp_d[p_][:, :])
                        else:
                            nc.gpsimd.collective_compute(
                                "AllGather",
                                mybir.AluOpType.bypass,
                                replica_groups=[list(range(CORES))],
                                ins=[h1sp_d[p_].ap().opt()],
                                outs=[h1pf_d[p_].ap().opt()],
                            )
                else:
                    nc.tensor.matmul(hp[:], lhsT=ones_t[:1, :],
                                     rhs=bias2_t[:1, :],
                                     start=False, stop=True)
                    ot = workp.tile([128, D], f16, tag="ot", name="ot")
                    nc.scalar.copy(out=ot[:], in_=hp[:])
                    nc.sync.dma_start(out=out_d[w * 128:(w + 1) * 128, :],
                                      in_=ot[:])

            ZG_BUFS = 16
            gci = 0
            for L in range(2):
                selfsrc = zself_t if L == 0 else h1stage

                def gather_group(h, w):
                    nonlocal gci
                    Mc = Ma if h == 0 else Mb
                    gb = (0 if h == 0 else WPC * Ma) + w * Mc
                    if L == 0:
                        src_ap = ztp_d[:, :] if h == 0 else ztp_d[P0ROWS:, :]
                    else:
                        src_ap = h1pf_d[h][:, :]
                    zg = workp.tile([128, Mc * D], f16, tag=f"zg{h}",
                                    bufs=ZG_BUFS, name=f"zg{h}")
                    if gci < 2 * ZG_BUFS:
                        # first rotation: clear undefined SBUF so trimmed
                        # pad slots multiply 0-masks with finite data
                        nc.vector.memset(zg[:], 0.0)
                    nc.gpsimd.dma_gather(
                        out_ap=zg[:].rearrange("p (b e) -> p b e", e=D),
                        in_ap=src_ap,
                        idxs_ap=idx_t[:, gb * 8:(gb + Mc) * 8],
                        num_idxs=Mc * 128,
                        num_idxs_reg=int(Pg[(0 if h == 0 else WPC) + w]),
                        elem_size=D,
                        queue_num=gci % 4,
                    )
                    gci += 1
                    stc = workp.tile([128, Mc * D], f8, tag=f"stc{h}",
                                     bufs=6, name=f"stc{h}")
                    eng = nc.sync if (h + w) % 2 == 0 else nc.scalar
                    eng.dma_start(
                        out=stc[:],
                        in_=masks_d[:, gb * 128:(gb + Mc) * 128],
                    )
                    return zg, stc

                def mask_matmuls(cur, h, zg, stc, first, last):
                    Mc = Ma if h == 0 else Mb
                    for jm in range(Mc):
                        nc.tensor.matmul(
                            cur[:], lhsT=zg[:, ts(jm, D)],
                            rhs=stc[:, ts(jm, D)],
                            start=(first and jm == 0),
                            stop=(last and jm == Mc - 1),
                        )

                def epilogue(w, cur):
                    aggT = workp.tile([128, D], f16, tag="aggT", bufs=3,
                                      name="aggT")
                    nc.vector.tensor_tensor(
                        out=aggT[:], in0=cur[:],
                        in1=dinvB_t[:, ts(w, D)], op=OP.mult)
                    transform(L, w, aggT)

                if L == 0:
                    # window-major: both pieces of a window back-to-back, all
                    # accumulation stays in PSUM; w=23's transform (and the
                    # piece-0 AllGather) fires ~halfway through layer 1
                    for w in range(WPC):
                        zg0, stc0 = gather_group(0, w)
                        zg1, stc1 = gather_group(1, w)
                        cur = psump.tile([128, D], f32, tag="agg",
                                         bufs=4, name="agg")
                        nc.tensor.matmul(
                            cur[:], lhsT=selfsrc[:, ts(w, D)],
                            rhs=diag_t[:, ts(w, D)],
                            start=True, stop=False)
                        mask_matmuls(cur, 0, zg0, stc0, first=False,
                                     last=False)
                        mask_matmuls(cur, 1, zg1, stc1, first=False,
                                     last=True)
                        epilogue(w, cur)
                else:
                    # phase-major: piece-0 gathers overlap the piece-1
                    # AllGather; partial sums round-trip through aggbuf
                    for w in range(WPC):
                        zg0, stc0 = gather_group(0, w)
                        cur = psump.tile([128, D], f32, tag="agg",
                                         bufs=4, name="agg")
                        nc.tensor.matmul(
                            cur[:], lhsT=selfsrc[:, ts(w, D)],
                            rhs=diag_t[:, ts(w, D)],
                            start=True, stop=False)
                        mask_matmuls(cur, 0, zg0, stc0, first=False,
                                     last=True)
                        nc.scalar.copy(out=aggbuf[:, ts(w, D)], in_=cur[:])
                    for w in range(WPC):
                        zg1, stc1 = gather_group(1, w)
                        cur = psump.tile([128, D], f32, tag="agg",
                                         bufs=4, name="agg")
                        mask_matmuls(cur, 1, zg1, stc1, first=True,
                                     last=True)
                        asum = workp.tile([128, D], f32, tag="asum",
                                          bufs=3, name="asum")
                        nc.vector.tensor_tensor(
                            out=asum[:], in0=cur[:],
                            in1=aggbuf[:, ts(w, D)], op=OP.add)
                        epilogue(w, asum)

    nc.finalize()
    _BUILD_CACHE[key] = nc
    return nc


def _temb_table(Wt1, bt1, Wt2, bt2, b1):
    """Exact time-MLP for all possible t values: [TMAX, D] float64 -> f32."""
    try:
        from scipy.special import erf
    except ImportError:
        import math
        erf = np.vectorize(math.erf)
    tv = np.arange(TMAX, dtype=np.float64)[:, None]          # [TMAX, 1]
    pre = tv @ np.asarray(Wt1, np.float64).T + np.asarray(bt1, np.float64)
    g = 0.5 * pre * (1.0 + erf(pre / np.sqrt(2.0)))
    emb = g @ np.asarray(Wt2, np.float64).T + np.asarray(bt2, np.float64)
    emb = emb + np.asarray(b1, np.float64)
    return emb.astype(np.float32)


def kernel(z, edge_index, t, Wt1, bt1, Wt2, bt2, W1, b1, W2, b2):
    z = np.asarray(z, dtype=np.float32)
    t = np.asarray(t)
    Ms, tok, inv_tok, dinv, core_inputs = _prep(z, edge_index, t)
    nc = _build(Ms)

    table = _temb_table(Wt1, bt1, Wt2, bt2, b1)

    # z table in piece-major row order (dinv-scaled); spare rows stay zero
    ztab = np.zeros((NTOK, D), dtype=np.float16)
    ztab[tok] = (z * dinv[:, None]).astype(np.float16)
    T = np.arange(NTOK)
    k_ = T // SHARD
    wl_ = (T % SHARD) >> 7
    off_ = T & 127
    p_ = (wl_ >= PB[1]).astype(np.int64)
    pw_ = np.where(p_ == 0, PB[1], WPC - PB[1])
    prow = (k_ * (pw_ * 128) + (wl_ - p_ * PB[1]) * 128 + off_
            + p_ * P0ROWS)
    ztp = np.zeros((NTOK, D), dtype=np.float16)
    ztp[prow] = ztab
    del ztab

    shared = {
        "ztp": ztp,
        "w1t": np.ascontiguousarray(np.asarray(W1, np.float32).T).astype(np.float16),
        "w2t": np.ascontiguousarray(np.asarray(W2, np.float32).T).astype(np.float16),
        "bias2": np.asarray(b2, np.float32).astype(np.float16).reshape(1, D),
        "ones": np.ones((1, D), dtype=np.float16),
    }

    in_maps = []
    for kk in range(CORES):
        ci = core_inputs[kk]
        m = dict(shared)
        m["maskT"] = ci["maskT"]
        m["idx16"] = ci["idx16"]
        m["diagT"] = ci["diagT"]
        m["zself"] = ci["zself"]
        m["dinvA"] = ci["dinvA"]
        m["dinvB"] = ci["dinvB"]
        own = ci["_own"]
        te = np.zeros((SHARD, D), dtype=np.float16)
        ok = own >= 0
        te[ok] = table[np.asarray(t[own[ok]], np.int64)].astype(np.float16)
        m["temb"] = np.ascontiguousarray(
            te.reshape(WPC, 128, D).transpose(1, 0, 2).reshape(128, SHARD))
        in_maps.append(m)

    from concourse.bass_utils import run_bass_kernel_spmd
    res = run_bass_kernel_spmd(nc, in_maps, core_ids=list(range(CORES)))
    global LAST_RESULT
    LAST_RESULT = res
    out_tok = np.concatenate(
        [res.results[kk]["outshard"] for kk in range(CORES)], axis=0)
    return out_tok[tok].astype(np.float32)


# revision 32
# speedup vs baseline: 1.0106x; 1.0106x over previous
"""GCN denoise net (2-layer GCNConv + time MLP) on 8 Trainium2 NeuronCores.

Strategy (v4 — unified piece schedule, fp8 masks, separable-norm folding):
  - Aggregate-then-transform: out = (A_hat @ x) @ W.T + b, exploiting linearity.
  - A_hat = D^-1/2 (A+I) D^-1/2 is separable: dinv[src] is folded into the
    gathered token table (host pre-scales), dinv[tgt] is applied once per
    target window after aggregation.  Edge "stiles" are pure 0/1 one-hots
    streamed as fp8 (exact).
  - ONE schedule for both layers: edges grouped by (source piece, target
    window), where pieces are the h1 AllGather pieces (windows [0,24) and
    [24,49) of every core).  The z table is laid out in the SAME piece-major
    row order the AllGather produces, so layer 1 and layer 2 share one fp8
    mask stream and one gather-index stream; only the gather source tensor
    differs (z table vs gathered h1).
  - Nodes permuted into 50176 "token" slots (392 windows of 128) with a
    degree-balanced serpentine assignment; per-(piece, window) edge groups
    fit Ma/Mb blocks of 128.
  - Targets sharded: core c owns windows [c*49, (c+1)*49).
  - Edge aggregation per 128-target window via PE one-hot matmuls
    (f16 gathered features x fp8 masks, accumulated in PSUM).
  - Source features fetched by gpsimd dma_gather (1024-idx calls, HW max);
    self-loops are one dense matmul per window (identity diag) against the
    own-shard tile.
  - temb (time-MLP table, b1 folded) preloaded once in wrapped layout.
  - h1 (dinv-scaled on-chip) exchanged between layers with a two-piece
    on-device AllGather; own h1 windows stay resident in SBUF for layer-2
    self matmuls.
"""

import sys
import numpy as np

sys.path.insert(0, "/opt/trn_rl_repo")

# ---------------- problem constants (hardcoded per contract) ----------------
N = 50000
E = 640000
D = 128
CORES = 8
NW = 392                      # 128-token windows total
NTOK = NW * 128               # 50176 padded tokens
WPC = NW // CORES             # 49 windows per core
SHARD = WPC * 128             # 6272 tokens per core
G = 8                         # gather chunk size in blocks (1024 idxs = HW max)
TMAX = 1000
PB = [0, 24, WPC]             # AllGather piece boundaries (windows per core);
                              # both pieces must stay <= 31 windows for int16
P0ROWS = CORES * PB[1] * 128  # 24576 rows in piece 0


def _prep(z, edge_index, t):
    """Host preprocessing: degrees, dinv, balanced permutation, one shared
    (piece, window) edge schedule, fp8 one-hot masks."""
    import ml_dtypes
    f8 = ml_dtypes.float8_e4m3

    row = np.asarray(edge_index[0], dtype=np.int64)
    col = np.asarray(edge_index[1], dtype=np.int64)

    deg = np.bincount(col, minlength=N).astype(np.float64) + 1.0
    dinv = (1.0 / np.sqrt(deg)).astype(np.float32)

    # --- balanced node->token permutation (serpentine over in-degree) ---
    indeg = np.bincount(col, minlength=N)
    order = np.argsort(-indeg, kind="stable")
    i = np.arange(N)
    r = i // NW
    j = i % NW
    win = np.where(r % 2 == 0, j, NW - 1 - j)
    token = win * 128 + r
    tok = np.empty(N, dtype=np.int64)
    tok[order] = token
    inv_tok = np.full(NTOK, -1, dtype=np.int64)
    inv_tok[tok] = np.arange(N)

    tr = tok[row]
    tc = tok[col]

    # piece-major row index of each source token (matches AllGather layout):
    # piece0: rows k*3072 + wl*128 + off (wl < 24), piece1 relative rows
    k_ = tr // SHARD
    wl_ = (tr % SHARD) >> 7
    off_ = tr & 127
    src_piece = (wl_ >= PB[1]).astype(np.int64)
    pw_ = np.where(src_piece == 0, PB[1], WPC - PB[1])
    src_row = k_ * (pw_ * 128) + (wl_ - src_piece * PB[1]) * 128 + off_

    core_of = tc // SHARD
    per_core = []
    Ma = 1
    Mb = 1
    for k in range(CORES):
        sel = core_of == k
        tck = tc[sel]
        wloc = (tck - k * SHARD) >> 7
        coff = tck & 127
        key = src_piece[sel] * WPC + wloc
        o2 = np.argsort(key, kind="stable")
        sidx = src_row[sel][o2]
        coffo = coff[o2]
        cnt = np.bincount(key[o2], minlength=2 * WPC)
        Ma = max(Ma, int(np.ceil(cnt[:WPC].max() / 128)))
        Mb = max(Mb, int(np.ceil(cnt[WPC:].max() / 128)))
        per_core.append((sidx, coffo, cnt))

    NBT = WPC * (Ma + Mb)     # total blocks in the shared schedule
    # per-group valid-idx count, uniform across cores (goes in num_idxs_reg):
    # cores with fewer edges pad with zero-row idxs up to Pg, then -1 beyond
    Pg = np.max(np.stack([pc[2] for pc in per_core]), axis=0)   # [2*WPC]

    core_inputs = []
    for k in range(CORES):
        sidx, coff, cnt = per_core[k]
        # pad slots get idx -1: they trail within each (piece, window) group,
        # and gather calls are group-aligned, so the gather ucode trims them
        # before descriptor generation (no Q7/SDMA cost).
        s_arr = np.full(NBT * 128, -1, dtype=np.int16)
        c_arr = np.full(NBT * 128, -1, dtype=np.int64)
        starts = np.concatenate([[0], np.cumsum(cnt)])
        for g in range(2 * WPC):
            m = int(cnt[g])
            if g < WPC:
                b0 = g * Ma * 128
            else:
                b0 = (WPC * Ma + (g - WPC) * Mb) * 128
            if m > 0:
                src = slice(starts[g], starts[g] + m)
                s_arr[b0:b0 + m] = sidx[src].astype(np.int16)
                c_arr[b0:b0 + m] = coff[src]
            # zero-row pads up to the shared per-group count Pg[g]
            s_arr[b0 + m:b0 + int(Pg[g])] = 0

        # transposed fp8 0/1 masks: maskT[e, b*128 + t] = (coff(slot b,e)==t)
        st_flat = np.zeros((NBT * 128, 128), dtype=np.float32)
        val = c_arr >= 0
        st_flat[np.nonzero(val)[0], c_arr[val]] = 1.0
        maskT = np.ascontiguousarray(
            st_flat.reshape(NBT, 128, 128).transpose(1, 0, 2)
            .reshape(128, NBT * 128)).astype(f8)

        # wrapped gather-index layout: unwrapped[i] = tile[i % 16, i // 16],
        # replicated across the 8 groups of 16 partitions
        wr = np.ascontiguousarray(s_arr.reshape(NBT * 8, 16).T)  # [16, NBT*8]
        idx_t = np.ascontiguousarray(np.tile(wr, (8, 1)))        # [128, NBT*8]

        # self-loop identity diagonal [r, w*128+t]: 1.0 at own real tokens
        own = inv_tok[k * SHARD:(k + 1) * SHARD]                 # [6272]
        ok = own >= 0
        dg = np.where(ok, 1.0, 0.0).astype(np.float16)
        diagT = np.zeros((128, SHARD), dtype=np.float16)
        p = np.arange(SHARD)
        diagT[p & 127, (p >> 7) * 128 + (p & 127)] = dg

        # own z rows, dinv-scaled, wrapped: zself[p, w*128+f]
        zself = np.zeros((128, SHARD), dtype=np.float16)
        zs = np.zeros((SHARD, D), dtype=np.float16)
        zs[ok] = (z[own[ok]] * dinv[own[ok]][:, None]).astype(np.float16)
        zself[:, :] = zs.reshape(WPC, 128, D).transpose(1, 0, 2).reshape(128, SHARD)

        # dinvA[p, w*128+f] = dinv[token(w*128+p)]  (h1 pre-scale, bcast on f)
        dv = np.zeros(SHARD, dtype=np.float32)
        dv[ok] = dinv[own[ok]]
        dinvA = np.ascontiguousarray(
            dv.reshape(WPC, 128, 1).repeat(D, axis=2)
            .transpose(1, 0, 2).reshape(128, SHARD)).astype(np.float16)
        # dinvB[p, w*128+t] = dinv[token(w*128+t)]  (agg post-scale, bcast on p)
        dinvB = np.ascontiguousarray(
            np.broadcast_to(dv, (128, SHARD))).astype(np.float16)

        core_inputs.append({
            "maskT": maskT,
            "idx16": idx_t,
            "diagT": diagT,
            "zself": zself,
            "dinvA": dinvA,
            "dinvB": dinvB,
            "_own": own,
        })
    return (Ma, Mb, tuple(int(x) for x in Pg)), tok, inv_tok, dinv, core_inputs


_BUILD_CACHE = {}
LAST_RESULT = None


def _build(Ms):
    """Build the SPMD Bass program (one NEFF shared by all 8 cores)."""
    Ma, Mb, Pg = Ms
    import os as _os
    no_cc = bool(int(_os.environ.get("KERNEL_NO_CC", "0")))
    key = (Ma, Mb, Pg, no_cc)
    if key in _BUILD_CACHE:
        return _BUILD_CACHE[key]

    import concourse.bass as bass
    import concourse.mybir as mybir
    import concourse.tile as tile
    from concourse.bass import ts

    f8 = mybir.dt.float8e4
    f16 = mybir.dt.float16
    f32 = mybir.dt.float32
    i16 = mybir.dt.int16
    AF = mybir.ActivationFunctionType
    OP = mybir.AluOpType

    NBT = WPC * (Ma + Mb)

    from concourse import bacc
    nc = bacc.Bacc(num_devices=CORES, num_swdge_queues=4)

    # z table in piece-major row order (same layout the AllGather produces)
    ztp_d = nc.dram_tensor("ztp", [NTOK, D], f16, kind="ExternalInput")
    idx_d = nc.dram_tensor("idx16", [128, NBT * 8], i16, kind="ExternalInput")
    masks_d = nc.dram_tensor("maskT", [128, NBT * 128], f8, kind="ExternalInput")
    temb_d = nc.dram_tensor("temb", [128, SHARD], f16, kind="ExternalInput")
    diag_d = nc.dram_tensor("diagT", [128, SHARD], f16, kind="ExternalInput")
    zself_d = nc.dram_tensor("zself", [128, SHARD], f16, kind="ExternalInput")
    dinvA_d = nc.dram_tensor("dinvA", [128, SHARD], f16, kind="ExternalInput")
    dinvB_d = nc.dram_tensor("dinvB", [128, SHARD], f16, kind="ExternalInput")
    w1t_d = nc.dram_tensor("w1t", [D, D], f16, kind="ExternalInput")
    w2t_d = nc.dram_tensor("w2t", [D, D], f16, kind="ExternalInput")
    bias2_d = nc.dram_tensor("bias2", [1, D], f16, kind="ExternalInput")
    ones_d = nc.dram_tensor("ones", [1, D], f16, kind="ExternalInput")

    out_d = nc.dram_tensor("outshard", [SHARD, D], f16, kind="ExternalOutput")
    h1sp_d = [nc.dram_tensor(f"h1sp{q}", [(PB[q + 1] - PB[q]) * 128, D], f16)
              for q in range(2)]
    h1pf_d = [nc.dram_tensor(f"h1pf{q}", [CORES * (PB[q + 1] - PB[q]) * 128, D],
                             f16, addr_space="Shared") for q in range(2)]

    with tile.TileContext(nc) as tc:
        with (
            tc.tile_pool(name="const", bufs=1) as constp,
            tc.tile_pool(name="work", bufs=2) as workp,
            tc.tile_pool(name="psum", bufs=2, space="PSUM") as psump,
        ):
            def load_const(dram, shape, dtype, eng=None):
                t_ = constp.tile(shape, dtype, name=dram.name + "_t")
                (eng or nc.sync).dma_start(out=t_[:], in_=dram[:, :])
                return t_

            idx_t = load_const(idx_d, [128, NBT * 8], i16)
            w1t_t = load_const(w1t_d, [D, D], f16)
            w2t_t = load_const(w2t_d, [D, D], f16)
            bias2_t = load_const(bias2_d, [1, D], f16)
            ones_t = load_const(ones_d, [1, D], f16)
            temb_t = load_const(temb_d, [128, SHARD], f16)
            diag_t = load_const(diag_d, [128, SHARD], f16)
            zself_t = load_const(zself_d, [128, SHARD], f16)
            dinvA_t = load_const(dinvA_d, [128, SHARD], f16)
            dinvB_t = load_const(dinvB_d, [128, SHARD], f16)

            aggbuf = constp.tile([128, SHARD], f32, name="aggbuf")
            h1stage = constp.tile([128, SHARD], f16, name="h1stage")

            def transform(L, w, aggT):
                # aggT: [f, t] f16; hp = aggT.T @ W.T -> [t, f']
                hp = psump.tile([128, D], f32, tag="hp", name="hp")
                wm = w1t_t if L == 0 else w2t_t
                nc.tensor.matmul(hp[:], lhsT=aggT[:], rhs=wm[:],
                                 start=True, stop=(L == 0))
                if L == 0:
                    # x = conv + temb ; h1 = elu(x) = relu(x)+exp(min(x,0))-1
                    x = workp.tile([128, D], f32, tag="x", name="x")
                    nc.vector.tensor_tensor(out=x[:], in0=hp[:],
                                            in1=temb_t[:, ts(w, D)], op=OP.add)
                    mn = workp.tile([128, D], f32, tag="mn", name="mn")
                    nc.vector.tensor_scalar(out=mn[:], in0=x[:],
                                            scalar1=0.0, scalar2=-60.0,
                                            op0=OP.min, op1=OP.max)
                    ex = workp.tile([128, D], f16, tag="ex", name="ex")
                    nc.scalar.activation(ex[:], mn[:], AF.Exp)
                    rl = workp.tile([128, D], f16, tag="rl", name="rl")
                    nc.scalar.activation(rl[:], x[:], AF.Relu)
                    e1 = workp.tile([128, D], f16, tag="e1", name="e1")
                    nc.vector.tensor_scalar(out=e1[:], in0=ex[:],
                                            scalar1=1.0, scalar2=None,
                                            op0=OP.subtract)
                    h1r = workp.tile([128, D], f16, tag="h1r", name="h1r")
                    nc.vector.tensor_tensor(out=h1r[:], in0=e1[:], in1=rl[:],
                                            op=OP.add)
                    # store dinv-scaled h1 (both for gathers and self matmuls)
                    nc.vector.tensor_tensor(out=h1stage[:, ts(w, D)],
                                            in0=h1r[:],
                                            in1=dinvA_t[:, ts(w, D)],
                                            op=OP.mult)
                    p_ = 0 if w < PB[1] else 1
                    lw = w - PB[p_]
                    nc.sync.dma_start(
                        out=h1sp_d[p_][lw * 128:(lw + 1) * 128, :],
                        in_=h1stage[:, ts(w, D)])
                    if w == PB[p_ + 1] - 1:
                        if no_cc:
                            rows_ = (PB[p_ + 1] - PB[p_]) * 128
                            nc.sync.dma_start(out=h1pf_d[p_][0:rows_, :],
                                              in_=h1sp_d[p_][:, :])
                        else:
                            nc.gpsimd.collective_compute(
                                "AllGather",
                                mybir.AluOpType.bypass,
                                replica_groups=[list(range(CORES))],
                                ins=[h1sp_d[p_].ap().opt()],
                                outs=[h1pf_d[p_].ap().opt()],
                            )
                else:
                    nc.tensor.matmul(hp[:], lhsT=ones_t[:1, :],
                                     rhs=bias2_t[:1, :],
                                     start=False, stop=True)
                    ot = workp.tile([128, D], f16, tag="ot", name="ot")
                    nc.scalar.copy(out=ot[:], in_=hp[:])
                    nc.sync.dma_start(out=out_d[w * 128:(w + 1) * 128, :],
                                      in_=ot[:])

            ZG_BUFS = 16
            gci = 0
            for L in range(2):
                selfsrc = zself_t if L == 0 else h1stage

                def gather_group(h, w):
                    nonlocal gci
                    Mc = Ma if h == 0 else Mb
                    gb = (0 if h == 0 else WPC * Ma) + w * Mc
                    if L == 0:
                        src_ap = ztp_d[:, :] if h == 0 else ztp_d[P0ROWS:, :]
                    else:
                        src_ap = h1pf_d[h][:, :]
                    zg = workp.tile([128, Mc * D], f16, tag=f"zg{h}",
                                    bufs=ZG_BUFS, name=f"zg{h}")
                    if gci < 2 * ZG_BUFS:
                        # first rotation: clear undefined SBUF so trimmed
                        # pad slots multiply 0-masks with finite data
                        nc.vector.memset(zg[:], 0.0)
                    nc.gpsimd.dma_gather(
                        out_ap=zg[:].rearrange("p (b e) -> p b e", e=D),
                        in_ap=src_ap,
                        idxs_ap=idx_t[:, gb * 8:(gb + Mc) * 8],
                        num_idxs=Mc * 128,
                        num_idxs_reg=int(Pg[(0 if h == 0 else WPC) + w]),
                        elem_size=D,
                        queue_num=gci % 4,
                    )
                    gci += 1
                    stc = workp.tile([128, Mc * D], f8, tag=f"stc{h}",
                                     bufs=6, name=f"stc{h}")
                    eng = nc.sync if (h + w) % 2 == 0 else nc.scalar
                    eng.dma_start(
                        out=stc[:],
                        in_=masks_d[:, gb * 128:(gb + Mc) * 128],
                    )
                    return zg, stc

                def mask_matmuls(cur, h, zg, stc, first, last):
                    Mc = Ma if h == 0 else Mb
                    for jm in range(Mc):
                        nc.tensor.matmul(
                            cur[:], lhsT=zg[:, ts(jm, D)],
                            rhs=stc[:, ts(jm, D)],
                            start=(first and jm == 0),
                            stop=(last and jm == Mc - 1),
                        )

                def epilogue(w, cur):
                    aggT = workp.tile([128, D], f16, tag="aggT", bufs=3,
                                      name="aggT")
                    nc.vector.tensor_tensor(
                        out=aggT[:], in0=cur[:],
                        in1=dinvB_t[:, ts(w, D)], op=OP.mult)
                    transform(L, w, aggT)

                if L == 0:
                    # window-major: both pieces of a window back-to-back, all
                    # accumulation stays in PSUM; w=23's transform (and the
                    # piece-0 AllGather) fires ~halfway through layer 1
                    for w in range(WPC):
                        zg0, stc0 = gather_group(0, w)
                        zg1, stc1 = gather_group(1, w)
                        cur = psump.tile([128, D], f32, tag="agg",
                                         bufs=4, name="agg")
                        nc.tensor.matmul(
                            cur[:], lhsT=selfsrc[:, ts(w, D)],
                            rhs=diag_t[:, ts(w, D)],
                            start=True, stop=False)
                        mask_matmuls(cur, 0, zg0, stc0, first=False,
                                     last=False)
                        mask_matmuls(cur, 1, zg1, stc1, first=False,
                                     last=True)
                        epilogue(w, cur)
                else:
                    # phase-major: piece-0 gathers overlap the piece-1
                    # AllGather; partial sums round-trip through aggbuf
                    for w in range(WPC):
                        zg0, stc0 = gather_group(0, w)
                        cur = psump.tile([128, D], f32, tag="agg",
                                         bufs=4, name="agg")
                        nc.tensor.matmul(
                            cur[:], lhsT=selfsrc[:, ts(w, D)],
                            rhs=diag_t[:, ts(w, D)],
                            start=True, stop=False)
                        mask_matmuls(cur, 0, zg0, stc0, first=False,
                                     last=True)
                        nc.scalar.copy(out=aggbuf[:, ts(w, D)], in_=cur[:])
                    for w in range(WPC):
                        zg1, stc1 = gather_group(1, w)
                        cur = psump.tile([128, D], f32, tag="agg",
                                         bufs=4, name="agg")
                        mask_matmuls(cur, 1, zg1, stc1, first=True,
                                     last=True)
                        asum = workp.tile([128, D], f32, tag="asum",
                                          bufs=3, name="asum")
                        nc.vector.tensor_tensor(
                            out=asum[:], in0=cur[:],
                            in1=aggbuf[:, ts(w, D)], op=OP.add)
                        epilogue(w, asum)

    nc.finalize()
    _BUILD_CACHE[key] = nc
    return nc


def _temb_table(Wt1, bt1, Wt2, bt2, b1):
    """Exact time-MLP for all possible t values: [TMAX, D] float64 -> f32."""
    try:
        from scipy.special import erf
    except ImportError:
        import math
        erf = np.vectorize(math.erf)
    tv = np.arange(TMAX, dtype=np.float64)[:, None]          # [TMAX, 1]
    pre = tv @ np.asarray(Wt1, np.float64).T + np.asarray(bt1, np.float64)
    g = 0.5 * pre * (1.0 + erf(pre / np.sqrt(2.0)))
    emb = g @ np.asarray(Wt2, np.float64).T + np.asarray(bt2, np.float64)
    emb = emb + np.asarray(b1, np.float64)
    return emb.astype(np.float32)


def kernel(z, edge_index, t, Wt1, bt1, Wt2, bt2, W1, b1, W2, b2):
    z = np.asarray(z, dtype=np.float32)
    t = np.asarray(t)
    Ms, tok, inv_tok, dinv, core_inputs = _prep(z, edge_index, t)
    nc = _build(Ms)

    table = _temb_table(Wt1, bt1, Wt2, bt2, b1)

    # z table in piece-major row order (dinv-scaled); spare rows stay zero
    ztab = np.zeros((NTOK, D), dtype=np.float16)
    ztab[tok] = (z * dinv[:, None]).astype(np.float16)
    T = np.arange(NTOK)
    k_ = T // SHARD
    wl_ = (T % SHARD) >> 7
    off_ = T & 127
    p_ = (wl_ >= PB[1]).astype(np.int64)
    pw_ = np.where(p_ == 0, PB[1], WPC - PB[1])
    prow = (k_ * (pw_ * 128) + (wl_ - p_ * PB[1]) * 128 + off_
            + p_ * P0ROWS)
    ztp = np.zeros((NTOK, D), dtype=np.float16)
    ztp[prow] = ztab
    del ztab

    shared = {
        "ztp": ztp,
        "w1t": np.ascontiguousarray(np.asarray(W1, np.float32).T).astype(np.float16),
        "w2t": np.ascontiguousarray(np.asarray(W2, np.float32).T).astype(np.float16),
        "bias2": np.asarray(b2, np.float32).astype(np.float16).reshape(1, D),
        "ones": np.ones((1, D), dtype=np.float16),
    }

    in_maps = []
    for kk in range(CORES):
        ci = core_inputs[kk]
        m = dict(shared)
        m["maskT"] = ci["maskT"]
        m["idx16"] = ci["idx16"]
        m["diagT"] = ci["diagT"]
        m["zself"] = ci["zself"]
        m["dinvA"] = ci["dinvA"]
        m["dinvB"] = ci["dinvB"]
        own = ci["_own"]
        te = np.zeros((SHARD, D), dtype=np.float16)
        ok = own >= 0
        te[ok] = table[np.asarray(t[own[ok]], np.int64)].astype(np.float16)
        m["temb"] = np.ascontiguousarray(
            te.reshape(WPC, 128, D).transpose(1, 0, 2).reshape(128, SHARD))
        in_maps.append(m)

    from concourse.bass_utils import run_bass_kernel_spmd
    res = run_bass_kernel_spmd(nc, in_maps, core_ids=list(range(CORES)))
    global LAST_RESULT
    LAST_RESULT = res
    out_tok = np.concatenate(
        [res.results[kk]["outshard"] for kk in range(CORES)], axis=0)
    return out_tok[tok].astype(np.float32)
